# revision 9
# baseline (speedup 1.0000x reference)
"""Data-parallel FFLayer kernel for 8 TRN2 NeuronCores (Bass/Tile).

Computes  out = relu( (x / (||x||_2_row + 1e-4)) @ W.T + b )  for
x [16384, 2048], W [2048, 2048], b [2048], all float32.

Sharding (data-parallel): x is split along batch into 8 shards of
[2048, 2048]; W and b are replicated.  Host-side input staging (pure
layout permutations + dtype rounding the device consumes directly).

Mixed-precision k-split matmul: the PE is the bottleneck (the kernel
is matmul-issue-bound end to end), so the contraction dim is split
into a bf16 part (KB k-tiles of 128) and an fp8-e4m3 part (NF = 16-KB
k-tiles) that runs in DoubleRow perf mode -- two k-tiles contracted
per pass at 2x MAC throughput.  Error budget: fp8 quantization noise
contributes ~2.45e-2 * sqrt(NF/16) to the final rel-err (measured on
device: KB=8 -> 1.740e-2, KB=6 -> 1.945e-2, both matching the host
simulation to 4 digits); KB=6 cuts PE issue time to 11/16 = 0.6875x
of all-bf16 against the 2e-2 gate.

Both operands are pre-scaled on host by SX=16 (x) and SW=4096 (W) --
powers of two, so bf16/fp8 rounding is unchanged -- to put fp8 values
in e4m3's normal range.  The per-row output scale then becomes
s = 1/(SW*sqrt(nsq) + SX*SW*eps) where nsq = sum((SX*x)^2) comes from
the bf16 x stream (norm error from bf16 input ~1e-5, negligible).

Per-core pipeline, for each of 16 row-tiles:
  1. DMA xt16 (bf16 lhsT), xt8 (fp8 DR lhsT) and x_bf (bf16, norm) in
     (3 tiles ahead, so a congested DMA queue cannot stall the PE).
  2. ScalarE Square activation with accum_out -> row sum-of-squares;
     Sqrt with scale=SW^2; DVE +C*eps, reciprocal -> s [128,1].
  3. bf16 phase (KB k-tiles, 4 matmuls of 512 cols each) and fp8
     DoubleRow phase (NP pair-tiles x 4 matmuls) accumulate into the
     same PSUM chunks.  Phase order alternates per row-tile (even:
     bf16 then fp8, odd: fp8 then bf16) so the PE switches matmul
     dtype/mode once per tile instead of twice (~0.6us/tile measured
     switch cost).
  4. Eviction: DVE s-scale (per-partition scalar, PSUM->SBUF), DVE
     bias add, ScalarE ReLU, DMA out (fp32).
Emit order pipelines 3 tiles deep so the in-order ACT/DVE streams
never stall the PE.

Startup: W is staged chunk-contiguous ([KB, 4, 128, 512]) so each
512-col chunk is one sequential-DRAM DMA, and the first matmul is
gated on a 128KB chunk; tile 0's matmuls then chase the W stream.
Tail: the last row-tile groups its fp8 phase per 512-col chunk with
immediate eviction, so the kernel tail after the final matmul is one
chunk's evict+DMA instead of four.
"""

import numpy as np

B, IN, OUT, NCORES = 16384, 2048, 2048, 8
BS = B // NCORES  # batch rows per core
P = 128
NB = BS // P  # b-tiles per core
NK = IN // P  # k-tiles
KB = 6  # bf16 k-tiles; the remaining NF = NK-KB run in fp8 DoubleRow
NF = NK - KB
NP = NF // 2  # fp8 DoubleRow pair-tiles
EPS = 1e-4
SX = 16.0  # host pre-scale on x (power of 2: exact in bf16/fp8)
SW = 4096.0  # host pre-scale on W
CEPS = SX * SW * EPS

_NC_CACHE = {}


def _build_nc():
    import concourse.mybir as mybir
    import concourse.tile as tile
    from concourse import bacc

    f32 = mybir.dt.float32
    bf16 = mybir.dt.bfloat16
    fp8 = mybir.dt.float8e4
    AF = mybir.ActivationFunctionType
    DR = mybir.MatmulPerfMode.DoubleRow

    nc = bacc.Bacc()
    xb_d = nc.declare_dram_parameter("xb", [BS, IN], bf16, isOutput=False)
    xt_d = nc.declare_dram_parameter("xt", [P, NB, KB, P], bf16, isOutput=False)
    x8_d = nc.declare_dram_parameter("x8", [P, NB, NP, 2, P], fp8, isOutput=False)
    wt_d = nc.declare_dram_parameter("wt", [KB, 4, P, 512], bf16, isOutput=False)
    w8_d = nc.declare_dram_parameter("w8", [P, NP, 2, OUT], fp8, isOutput=False)
    b_d = nc.declare_dram_parameter("bias", [P, OUT], f32, isOutput=False)
    out_d = nc.declare_dram_parameter("out", [BS, OUT], f32, isOutput=True)

    with tile.TileContext(nc) as tc:
        with (
            tc.tile_pool(name="wtb", bufs=1) as wtb,
            tc.tile_pool(name="consts", bufs=1) as consts,
            tc.tile_pool(name="xin", bufs=4) as xin,
            tc.tile_pool(name="xtp", bufs=4) as xtp,
            tc.tile_pool(name="x8p", bufs=4) as x8p,
            tc.tile_pool(name="sq", bufs=2) as sqp,
            tc.tile_pool(name="outp", bufs=6) as outp,
            tc.tile_pool(name="small", bufs=8) as small,
            tc.tile_pool(name="po", bufs=4, space="PSUM") as pop,
        ):
            bias_sb = consts.tile([P, OUT], f32)
            # wt_sb[ko][c]: one [128, 512] tile per (k-slice, col-chunk)
            # so the first matmul waits on a 128KB DMA, not 512KB
            wt_sb = [[None] * 4 for _ in range(KB)]
            # Warm the Square/Sqrt ACT tables while DMA streams in --
            # the lazy table load (1.3us) otherwise lands in the
            # middle of tile 0's norm chain.
            warm = consts.tile([P, 1], f32)
            nc.vector.memset(warm, 1.0)
            nc.scalar.activation(out=warm, in_=warm, func=AF.Square)
            nc.scalar.activation(out=warm, in_=warm, func=AF.Sqrt)

            def load_wt(ko):
                for c in range(4):
                    tb = wtb.tile([P, 512], bf16, tag=f"wt{ko}_{c}",
                                  name=f"wt{ko}_{c}")
                    nc.sync.dma_start(tb, wt_d[ko, c])
                    wt_sb[ko][c] = tb

            def load_xt(bt):
                xt_sb = xtp.tile([P, KB, P], bf16, name=f"xt{bt}", tag="xt")
                nc.sync.dma_start(xt_sb, xt_d[:, bt])
                x8_sb = x8p.tile([P, NP, 2, P], fp8, name=f"x8{bt}", tag="x8")
                nc.sync.dma_start(x8_sb, x8_d[:, bt])
                return xt_sb, x8_sb

            def load_x(bt):
                x_t = xin.tile([P, IN], bf16, name=f"x{bt}", tag="x")
                nc.sync.dma_start(x_t, xb_d[bt * P : (bt + 1) * P, :])
                return x_t

            def stage_load(bt):
                """DMA the matmul lhsT tiles and the norm tile for bt."""
                xt_sb, x8_sb = load_xt(bt)
                return xt_sb, x8_sb, load_x(bt)

            def stage_norm(st):
                """Row sum-of-squares of SX*x -> s = 1/(SW*sqrt+C*eps),
                off the PE critical path (only eviction consumes s)."""
                _, _, x_t = st
                sq = sqp.tile([P, IN], f32)
                nsq = small.tile([P, 1], f32)
                nc.scalar.activation(
                    out=sq, in_=x_t, func=AF.Square, accum_out=nsq
                )
                nrm = small.tile([P, 1], f32)
                # sqrt(nsq * SW^2) = SW * SX * ||x||
                nc.scalar.activation(
                    out=nrm, in_=nsq, func=AF.Sqrt, scale=SW * SW
                )
                nc.vector.tensor_scalar_add(nrm, nrm, CEPS)
                s = small.tile([P, 1], f32)
                nc.vector.reciprocal(s, nrm)
                return s

            def mm_bf16_chunk(xt_sb, ps, h, n2, ko, start, stop):
                nc.tensor.matmul(
                    ps[h][:, n2 * 512 : (n2 + 1) * 512],
                    lhsT=xt_sb[:, ko, :],
                    rhs=wt_sb[ko][h * 2 + n2],
                    start=start,
                    stop=stop,
                )

            def mm_fp8_chunk(x8_sb, ps, h, n2, p, start, stop):
                c0 = h * 1024 + n2 * 512
                nc.tensor.matmul(
                    ps[h][:, n2 * 512 : (n2 + 1) * 512],
                    lhsT=x8_sb[:, p, :, :],
                    rhs=w8_sb[:, p, :, c0 : c0 + 512],
                    start=start,
                    stop=stop,
                    perf_mode=DR,
                )

            def new_ps():
                return [
                    pop.tile([P, 1024], f32, name=f"ps{h}", tag="ps")
                    for h in range(2)
                ]

            def stage_mm_bf16(st, ps, first, last):
                # ko-major: each lhsT weight load feeds 4 consecutive
                # matmuls; W chunk tiles arrive in the same order
                xt_sb, _, _ = st
                for ko in range(KB):
                    for h in range(2):
                        for n2 in range(2):
                            mm_bf16_chunk(
                                xt_sb, ps, h, n2, ko,
                                start=(first and ko == 0),
                                stop=(last and ko == KB - 1),
                            )
                return ps

            def stage_mm_fp8(st, ps, first, last):
                # fp8 DoubleRow: each pair-tile contracts two k-tiles
                # per pass at 2x MAC rate
                _, x8_sb, _ = st
                for p in range(NP):
                    for h in range(2):
                        for n2 in range(2):
                            mm_fp8_chunk(
                                x8_sb, ps, h, n2, p,
                                start=(first and p == 0),
                                stop=(last and p == NP - 1),
                            )
                return ps

            def evict_chunk(bt, ps, s, h, n2, on_act):
                """Scale+bias+relu+store one [128,512] chunk."""
                lo = n2 * 512
                o_sb = outp.tile([P, 512], f32, tag="o_sb")
                if on_act:
                    nc.scalar.activation(
                        o_sb, ps[h][:, lo : lo + 512], AF.Copy, scale=s
                    )
                else:
                    nc.vector.tensor_scalar_mul(
                        o_sb, ps[h][:, lo : lo + 512], s
                    )
                nc.vector.tensor_add(
                    o_sb, o_sb, bias_sb[:, h * 1024 + lo : h * 1024 + lo + 512]
                )
                nc.scalar.activation(o_sb, o_sb, AF.Relu)
                nc.sync.dma_start(
                    out_d[
                        bt * P : (bt + 1) * P,
                        h * 1024 + lo : h * 1024 + lo + 512,
                    ],
                    o_sb,
                )

            def stage_evict(bt, ps, s):
                for h in range(2):
                    for n2 in range(2):
                        evict_chunk(bt, ps, s, h, n2, on_act=False)

            def stage_last(st, s, bt, fp8_first):
                """Last row-tile: the closing phase runs per 512-col
                chunk with immediate eviction, so the kernel tail after
                the final matmul is one chunk's evict+DMA instead of
                four.  The h=1 chunks scale on ACT so the tail DVE/ACT
                work runs in parallel."""
                xt_sb, x8_sb, _ = st
                ps = new_ps()
                if fp8_first:
                    stage_mm_fp8(st, ps, first=True, last=False)
                    for h in range(2):
                        for n2 in range(2):
                            for ko in range(KB):
                                mm_bf16_chunk(
                                    xt_sb, ps, h, n2, ko,
                                    start=False, stop=(ko == KB - 1),
                                )
                            evict_chunk(bt, ps, s, h, n2, on_act=(h == 1))
                else:
                    stage_mm_bf16(st, ps, first=True, last=False)
                    for h in range(2):
                        for n2 in range(2):
                            for p in range(NP):
                                mm_fp8_chunk(
                                    x8_sb, ps, h, n2, p,
                                    start=False, stop=(p == NP - 1),
                                )
                            evict_chunk(bt, ps, s, h, n2, on_act=(h == 1))

            # 3-deep software pipeline; see docstring.  DMA priority
            # order at startup: lead lhsT tiles and the W streams
            # (matmul gates), then bias and the lead norm tiles.
            xt0 = load_xt(0)
            for ko in range(KB):
                load_wt(ko)
            xt1 = load_xt(1)
            x0 = load_x(0)
            w8_sb = wtb.tile([P, NP, 2, OUT], fp8, tag="w8", name="w8")
            for p in range(NP):
                nc.sync.dma_start(w8_sb[:, p], w8_d[:, p])
            nc.sync.dma_start(bias_sb, b_d[:])
            states = {0: (*xt0, x0), 1: (*xt1, load_x(1))}
            states[2] = stage_load(2)
            scales = {0: stage_norm(states[0])}
            for bt in range(NB):
                fp8_first = bool(bt % 2)
                if bt == NB - 1:
                    stage_last(states[bt], scales[bt], bt, fp8_first)
                    del states[bt], scales[bt]
                    break
                ps = new_ps()
                if fp8_first:
                    stage_mm_fp8(states[bt], ps, first=True, last=False)
                else:
                    stage_mm_bf16(states[bt], ps, first=True, last=False)
                if bt + 1 < NB:
                    scales[bt + 1] = stage_norm(states[bt + 1])
                if fp8_first:
                    stage_mm_bf16(states[bt], ps, first=False, last=True)
                else:
                    stage_mm_fp8(states[bt], ps, first=False, last=True)
                if bt + 3 < NB:
                    states[bt + 3] = stage_load(bt + 3)
                stage_evict(bt, ps, scales[bt])
                del states[bt], scales[bt]

    nc.compile()
    return nc


def _get_nc():
    if "nc" not in _NC_CACHE:
        _NC_CACHE["nc"] = _build_nc()
    return _NC_CACHE["nc"]


def _make_in_maps(x, W, b):
    import ml_dtypes

    bf = ml_dtypes.bfloat16
    e4 = ml_dtypes.float8_e4m3

    x = np.asarray(x, dtype=np.float32)
    W = np.asarray(W, dtype=np.float32)
    b = np.asarray(b, dtype=np.float32)
    # host-side staging: layout permutations + the dtype rounding the
    # device matmul performs anyway (SX/SW are powers of two, so the
    # pre-scale commutes exactly with bf16/fp8 rounding)
    Ws = W.T * np.float32(SW)  # [IN, OUT] fp32
    # wt[ko, c] = bf16(SW*W.T)[ko*128:(ko+1)*128, c*512:(c+1)*512],
    # chunk-contiguous so each chunk DMA is a sequential DRAM walk
    wt = np.ascontiguousarray(
        Ws[: KB * P]
        .astype(bf)
        .reshape(KB, P, 4, 512)
        .transpose(0, 2, 1, 3)
    )
    # w8[ki, p, j, o] = fp8(SW * W.T)[(KB+2p+j)*128 + ki, o]
    w8 = np.ascontiguousarray(
        Ws[KB * P :].astype(e4).reshape(NP, 2, P, OUT).transpose(2, 0, 1, 3)
    )
    bias = np.ascontiguousarray(np.broadcast_to(b.reshape(1, OUT), (P, OUT)))
    in_maps = []
    for i in range(NCORES):
        xf = x[i * BS : (i + 1) * BS] * np.float32(SX)  # fp32
        xs = xf.astype(bf)
        # xt[ki, bt, ko, b] = xs[bt*128+b, ko*128+ki]  (blocked
        # transpose; per-partition-contiguous on device)
        xt = np.ascontiguousarray(
            xs[:, : KB * P].reshape(NB, P, KB, P).transpose(3, 0, 2, 1)
        )
        # x8[ki, bt, p, j, b] = fp8(SX*x)[bt*128+b, (KB+2p+j)*128+ki]
        x8 = np.ascontiguousarray(
            xf[:, KB * P :]
            .astype(e4)
            .reshape(NB, P, NP, 2, P)
            .transpose(4, 0, 2, 3, 1)
        )
        in_maps.append(
            {"xb": np.ascontiguousarray(xs), "xt": xt, "x8": x8,
             "wt": wt, "w8": w8, "bias": bias}
        )
    return in_maps


def _run(x, W, b, trace=False, tmpdir=None):
    from concourse.bass_utils import run_bass_kernel_spmd

    nc = _get_nc()
    res = run_bass_kernel_spmd(
        nc,
        _make_in_maps(x, W, b),
        core_ids=list(range(NCORES)),
        trace=trace,
        tmpdir=tmpdir,
    )
    out = np.concatenate(
        [np.asarray(res.results[i]["out"]) for i in range(NCORES)], axis=0
    )
    return out, res


def kernel(**inputs):
    out, _ = _run(inputs["x"], inputs["W"], inputs["b"])
    return out


def run_profiled(tmpdir=None, **inputs):
    out, res = _run(inputs["x"], inputs["W"], inputs["b"], trace=True, tmpdir=tmpdir)
    return out, res


# revision 14
# speedup vs baseline: 1.0365x; 1.0365x over previous
"""Data-parallel FFLayer kernel for 8 TRN2 NeuronCores (Bass/Tile).

Computes  out = relu( (x / (||x||_2_row + 1e-4)) @ W.T + b )  for
x [16384, 2048], W [2048, 2048], b [2048], all float32.

Sharding (data-parallel): x is split along batch into 8 shards of
[2048, 2048]; W and b are replicated.  Host-side input staging (pure
layout permutations + dtype rounding the device consumes directly).

Mixed-precision k-split matmul: the PE is the bottleneck (the kernel
is matmul-issue-bound end to end), so the contraction dim is split
into a bf16 part (KB k-tiles of 128) and an fp8-e4m3 part (NF = 16-KB
k-tiles) that runs in DoubleRow perf mode -- two k-tiles contracted
per pass at 2x MAC throughput.  Error budget: fp8 quantization noise
contributes ~2.45e-2 * sqrt(NF/16) to the final rel-err (measured on
device: KB=8 -> 1.740e-2, KB=6 -> 1.945e-2, both matching the host
simulation to 4 digits); KB=6 cuts PE issue time to 11/16 = 0.6875x
of all-bf16 against the 2e-2 gate.

Both operands are pre-scaled on host by SX=16 (x) and SW=4096 (W) --
powers of two, so bf16/fp8 rounding is unchanged -- to put fp8 values
in e4m3's normal range.  The per-row output scale then becomes
s = 1/(SW*sqrt(nsq) + SX*SW*eps) where nsq = sum((SX*x)^2) comes from
the bf16 x stream (norm error from bf16 input ~1e-5, negligible).

Per-core pipeline, for each of 16 row-tiles:
  1. DMA xt16 (bf16 lhsT), xt8 (fp8 DR lhsT) and x_bf (bf16, norm) in
     (3 tiles ahead, so a congested DMA queue cannot stall the PE).
  2. ScalarE Square activation with accum_out -> row sum-of-squares;
     Sqrt with scale=SW^2; DVE +C*eps, reciprocal -> s [128,1].
  3. bf16 phase (KB k-tiles, 4 matmuls of 512 cols each) and fp8
     DoubleRow phase (NP pair-tiles x 4 matmuls) accumulate into the
     same PSUM chunks.  Phase order alternates per row-tile (even:
     bf16 then fp8, odd: fp8 then bf16) so the PE switches matmul
     dtype/mode once per tile instead of twice (~0.6us/tile measured
     switch cost).
  4. Eviction: DVE s-scale (per-partition scalar, PSUM->SBUF), DVE
     bias add, ScalarE ReLU, DMA out (fp32).
Emit order pipelines 3 tiles deep so the in-order ACT/DVE streams
never stall the PE.

Startup: W is staged chunk-contiguous ([KB, 4, 128, 512]) so each
512-col chunk is one sequential-DRAM DMA, and the first matmul is
gated on a 128KB chunk; tile 0's matmuls then chase the W stream.
Tail: the last row-tile groups its fp8 phase per 512-col chunk with
immediate eviction, so the kernel tail after the final matmul is one
chunk's evict+DMA instead of four.
"""

import numpy as np

B, IN, OUT, NCORES = 16384, 2048, 2048, 8
BS = B // NCORES  # batch rows per core
P = 128
NB = BS // P  # b-tiles per core
NK = IN // P  # k-tiles
KB = 6  # bf16 k-tiles; the remaining NF = NK-KB run in fp8 DoubleRow
NF = NK - KB
NP = NF // 2  # fp8 DoubleRow pair-tiles
EPS = 1e-4
SX = 16.0  # host pre-scale on x (power of 2: exact in bf16/fp8)
SW = 4096.0  # host pre-scale on W
CEPS = SX * SW * EPS

_NC_CACHE = {}


def _build_nc():
    import concourse.mybir as mybir
    import concourse.tile as tile
    from concourse import bacc

    f32 = mybir.dt.float32
    bf16 = mybir.dt.bfloat16
    fp8 = mybir.dt.float8e4
    AF = mybir.ActivationFunctionType
    DR = mybir.MatmulPerfMode.DoubleRow

    nc = bacc.Bacc()
    xb_d = nc.declare_dram_parameter("xb", [BS, IN], bf16, isOutput=False)
    xt_d = nc.declare_dram_parameter("xt", [P, NB, KB, P], bf16, isOutput=False)
    x8_d = nc.declare_dram_parameter("x8", [P, NB, NP, 2, P], fp8, isOutput=False)
    # wt0: ko=0 chunk-contiguous (gates the first matmuls at 128KB
    # granularity); wtr: remaining k-slices as one 512KB DMA each
    # (the serial DMA-issue stream costs ~650ns per dma_start, so
    # fewer+bigger wins once the PE is running)
    wt0_d = nc.declare_dram_parameter("wt0", [4, P, 512], bf16, isOutput=False)
    wtr_d = nc.declare_dram_parameter(
        "wtr", [(KB - 1) * P, OUT], bf16, isOutput=False
    )
    w8_d = nc.declare_dram_parameter("w8", [P, NP, 2, OUT], fp8, isOutput=False)
    b_d = nc.declare_dram_parameter("bias", [P, OUT], f32, isOutput=False)
    out_d = nc.declare_dram_parameter("out", [BS, OUT], f32, isOutput=True)

    with tile.TileContext(nc) as tc:
        with (
            tc.tile_pool(name="wtb", bufs=1) as wtb,
            tc.tile_pool(name="consts", bufs=1) as consts,
            tc.tile_pool(name="xin", bufs=4) as xin,
            tc.tile_pool(name="xtp", bufs=4) as xtp,
            tc.tile_pool(name="x8p", bufs=4) as x8p,
            tc.tile_pool(name="sq", bufs=2) as sqp,
            tc.tile_pool(name="outp", bufs=6) as outp,
            tc.tile_pool(name="small", bufs=8) as small,
            tc.tile_pool(name="po", bufs=4, space="PSUM") as pop,
        ):
            bias_sb = consts.tile([P, OUT], f32)
            # wt_sb[ko][c]: one [128, 512] tile per (k-slice, col-chunk)
            # so the first matmul waits on a 128KB DMA, not 512KB
            wt_sb = [[None] * 4 for _ in range(KB)]
            # Warm the Square/Sqrt ACT tables while DMA streams in --
            # the lazy table load (1.3us) otherwise lands in the
            # middle of tile 0's norm chain.
            warm = consts.tile([P, 1], f32)
            nc.vector.memset(warm, 1.0)
            nc.scalar.activation(out=warm, in_=warm, func=AF.Square)
            nc.scalar.activation(out=warm, in_=warm, func=AF.Sqrt)

            def load_wt(ko):
                if ko == 0:
                    for c in range(4):
                        tb = wtb.tile([P, 512], bf16, tag=f"wt0_{c}",
                                      name=f"wt0_{c}")
                        nc.sync.dma_start(tb, wt0_d[c])
                        wt_sb[0][c] = tb
                else:
                    tb = wtb.tile([P, OUT], bf16, tag=f"wt{ko}",
                                  name=f"wt{ko}")
                    nc.sync.dma_start(
                        tb, wtr_d[(ko - 1) * P : ko * P, :]
                    )
                    for c in range(4):
                        wt_sb[ko][c] = tb[:, c * 512 : (c + 1) * 512]

            def load_xt(bt):
                xt_sb = xtp.tile([P, KB, P], bf16, name=f"xt{bt}", tag="xt")
                nc.sync.dma_start(xt_sb, xt_d[:, bt])
                x8_sb = x8p.tile([P, NP, 2, P], fp8, name=f"x8{bt}", tag="x8")
                nc.sync.dma_start(x8_sb, x8_d[:, bt])
                return xt_sb, x8_sb

            def load_x(bt):
                x_t = xin.tile([P, IN], bf16, name=f"x{bt}", tag="x")
                nc.sync.dma_start(x_t, xb_d[bt * P : (bt + 1) * P, :])
                return x_t

            def stage_load(bt):
                """DMA the matmul lhsT tiles and the norm tile for bt."""
                xt_sb, x8_sb = load_xt(bt)
                return xt_sb, x8_sb, load_x(bt)

            def stage_norm(st):
                """Row sum-of-squares of SX*x -> s = 1/(SW*sqrt+C*eps),
                off the PE critical path (only eviction consumes s)."""
                _, _, x_t = st
                sq = sqp.tile([P, IN], f32)
                nsq = small.tile([P, 1], f32)
                nc.scalar.activation(
                    out=sq, in_=x_t, func=AF.Square, accum_out=nsq
                )
                nrm = small.tile([P, 1], f32)
                # sqrt(nsq * SW^2) = SW * SX * ||x||
                nc.scalar.activation(
                    out=nrm, in_=nsq, func=AF.Sqrt, scale=SW * SW
                )
                nc.vector.tensor_scalar_add(nrm, nrm, CEPS)
                s = small.tile([P, 1], f32)
                nc.vector.reciprocal(s, nrm)
                return s

            def mm_bf16_chunk(xt_sb, ps, h, n2, ko, start, stop):
                nc.tensor.matmul(
                    ps[h][:, n2 * 512 : (n2 + 1) * 512],
                    lhsT=xt_sb[:, ko, :],
                    rhs=wt_sb[ko][h * 2 + n2],
                    start=start,
                    stop=stop,
                )

            def mm_fp8_chunk(x8_sb, ps, h, n2, p, start, stop):
                c0 = h * 1024 + n2 * 512
                nc.tensor.matmul(
                    ps[h][:, n2 * 512 : (n2 + 1) * 512],
                    lhsT=x8_sb[:, p, :, :],
                    rhs=w8_sb[:, p, :, c0 : c0 + 512],
                    start=start,
                    stop=stop,
                    perf_mode=DR,
                )

            def new_ps():
                return [
                    pop.tile([P, 1024], f32, name=f"ps{h}", tag="ps")
                    for h in range(2)
                ]

            def stage_mm_bf16(st, ps, first, last):
                # ko-major: each lhsT weight load feeds 4 consecutive
                # matmuls; W chunk tiles arrive in the same order
                xt_sb, _, _ = st
                for ko in range(KB):
                    for h in range(2):
                        for n2 in range(2):
                            mm_bf16_chunk(
                                xt_sb, ps, h, n2, ko,
                                start=(first and ko == 0),
                                stop=(last and ko == KB - 1),
                            )
                return ps

            def stage_mm_fp8(st, ps, first, last):
                # fp8 DoubleRow: each pair-tile contracts two k-tiles
                # per pass at 2x MAC rate
                _, x8_sb, _ = st
                for p in range(NP):
                    for h in range(2):
                        for n2 in range(2):
                            mm_fp8_chunk(
                                x8_sb, ps, h, n2, p,
                                start=(first and p == 0),
                                stop=(last and p == NP - 1),
                            )
                return ps

            def evict_chunk(bt, ps, s, h, n2, on_act):
                """Scale+bias+relu+store one [128,512] chunk.  The out
                DMA issues from the ACT engine (also a HWDGE issuer),
                right after its relu in ACT program order -- this keeps
                the serial Sync DMA stream free for input loads."""
                lo = n2 * 512
                o_sb = outp.tile([P, 512], f32, tag="o_sb")
                if on_act:
                    nc.scalar.activation(
                        o_sb, ps[h][:, lo : lo + 512], AF.Copy, scale=s
                    )
                else:
                    nc.vector.tensor_scalar_mul(
                        o_sb, ps[h][:, lo : lo + 512], s
                    )
                nc.vector.tensor_add(
                    o_sb, o_sb, bias_sb[:, h * 1024 + lo : h * 1024 + lo + 512]
                )
                nc.scalar.activation(o_sb, o_sb, AF.Relu)
                nc.scalar.dma_start(
                    out_d[
                        bt * P : (bt + 1) * P,
                        h * 1024 + lo : h * 1024 + lo + 512,
                    ],
                    o_sb,
                )

            def stage_evict(bt, ps, s):
                """Mid-tile eviction: one [128,1024] half per h, relu'd
                and stored as a single ACT-issued DMA per half."""
                for h in range(2):
                    o_sb = outp.tile([P, 1024], f32, tag="o_half")
                    for n2 in range(2):
                        lo = n2 * 512
                        nc.vector.tensor_scalar_mul(
                            o_sb[:, lo : lo + 512], ps[h][:, lo : lo + 512], s
                        )
                        nc.vector.tensor_add(
                            o_sb[:, lo : lo + 512],
                            o_sb[:, lo : lo + 512],
                            bias_sb[:, h * 1024 + lo : h * 1024 + lo + 512],
                        )
                    nc.scalar.activation(o_sb, o_sb, AF.Relu)
                    nc.scalar.dma_start(
                        out_d[bt * P : (bt + 1) * P, h * 1024 : (h + 1) * 1024],
                        o_sb,
                    )

            def stage_last(st, s, bt, fp8_first):
                """Last row-tile: the closing phase runs per 512-col
                chunk with immediate eviction, so the kernel tail after
                the final matmul is one chunk's evict+DMA instead of
                four.  The h=1 chunks scale on ACT so the tail DVE/ACT
                work runs in parallel."""
                xt_sb, x8_sb, _ = st
                ps = new_ps()
                if fp8_first:
                    stage_mm_fp8(st, ps, first=True, last=False)
                    for h in range(2):
                        for n2 in range(2):
                            for ko in range(KB):
                                mm_bf16_chunk(
                                    xt_sb, ps, h, n2, ko,
                                    start=False, stop=(ko == KB - 1),
                                )
                            evict_chunk(bt, ps, s, h, n2, on_act=(h == 1))
                else:
                    stage_mm_bf16(st, ps, first=True, last=False)
                    for h in range(2):
                        for n2 in range(2):
                            for p in range(NP):
                                mm_fp8_chunk(
                                    x8_sb, ps, h, n2, p,
                                    start=False, stop=(p == NP - 1),
                                )
                            evict_chunk(bt, ps, s, h, n2, on_act=(h == 1))

            # 3-deep software pipeline; see docstring.  DMA priority
            # order at startup: lead lhsT tiles and the W streams
            # (matmul gates), then bias and the lead norm tiles.
            xt0 = load_xt(0)
            for ko in range(KB):
                load_wt(ko)
            xt1 = load_xt(1)
            x0 = load_x(0)
            w8_sb = wtb.tile([P, NP, 2, OUT], fp8, tag="w8", name="w8")
            for p in range(NP):
                nc.sync.dma_start(w8_sb[:, p], w8_d[:, p])
            nc.sync.dma_start(bias_sb, b_d[:])
            states = {0: (*xt0, x0), 1: (*xt1, load_x(1))}
            states[2] = stage_load(2)
            scales = {0: stage_norm(states[0])}
            for bt in range(NB):
                fp8_first = bool(bt % 2)
                if bt == NB - 1:
                    stage_last(states[bt], scales[bt], bt, fp8_first)
                    del states[bt], scales[bt]
                    break
                ps = new_ps()
                if fp8_first:
                    stage_mm_fp8(states[bt], ps, first=True, last=False)
                else:
                    stage_mm_bf16(states[bt], ps, first=True, last=False)
                if bt + 1 < NB:
                    scales[bt + 1] = stage_norm(states[bt + 1])
                if fp8_first:
                    stage_mm_bf16(states[bt], ps, first=False, last=True)
                else:
                    stage_mm_fp8(states[bt], ps, first=False, last=True)
                if bt + 3 < NB:
                    states[bt + 3] = stage_load(bt + 3)
                stage_evict(bt, ps, scales[bt])
                del states[bt], scales[bt]

    nc.compile()
    return nc


def _get_nc():
    if "nc" not in _NC_CACHE:
        _NC_CACHE["nc"] = _build_nc()
    return _NC_CACHE["nc"]


def _make_in_maps(x, W, b):
    import ml_dtypes

    bf = ml_dtypes.bfloat16
    e4 = ml_dtypes.float8_e4m3

    x = np.asarray(x, dtype=np.float32)
    W = np.asarray(W, dtype=np.float32)
    b = np.asarray(b, dtype=np.float32)
    # host-side staging: layout permutations + the dtype rounding the
    # device matmul performs anyway (SX/SW are powers of two, so the
    # pre-scale commutes exactly with bf16/fp8 rounding)
    Ws = W.T * np.float32(SW)  # [IN, OUT] fp32
    # wt0[c] = bf16(SW*W.T)[0:128, c*512:(c+1)*512], chunk-contiguous
    # so each 128KB chunk DMA is a sequential DRAM walk; wtr = the
    # remaining bf16 k-slices in natural layout (one 512KB DMA each)
    wt16 = Ws[: KB * P].astype(bf)
    wt0 = np.ascontiguousarray(
        wt16[:P].reshape(P, 4, 512).transpose(1, 0, 2)
    )
    wtr = np.ascontiguousarray(wt16[P:])
    # w8[ki, p, j, o] = fp8(SW * W.T)[(KB+2p+j)*128 + ki, o]
    w8 = np.ascontiguousarray(
        Ws[KB * P :].astype(e4).reshape(NP, 2, P, OUT).transpose(2, 0, 1, 3)
    )
    bias = np.ascontiguousarray(np.broadcast_to(b.reshape(1, OUT), (P, OUT)))
    in_maps = []
    for i in range(NCORES):
        xf = x[i * BS : (i + 1) * BS] * np.float32(SX)  # fp32
        xs = xf.astype(bf)
        # xt[ki, bt, ko, b] = xs[bt*128+b, ko*128+ki]  (blocked
        # transpose; per-partition-contiguous on device)
        xt = np.ascontiguousarray(
            xs[:, : KB * P].reshape(NB, P, KB, P).transpose(3, 0, 2, 1)
        )
        # x8[ki, bt, p, j, b] = fp8(SX*x)[bt*128+b, (KB+2p+j)*128+ki]
        x8 = np.ascontiguousarray(
            xf[:, KB * P :]
            .astype(e4)
            .reshape(NB, P, NP, 2, P)
            .transpose(4, 0, 2, 3, 1)
        )
        in_maps.append(
            {"xb": np.ascontiguousarray(xs), "xt": xt, "x8": x8,
             "wt0": wt0, "wtr": wtr, "w8": w8, "bias": bias}
        )
    return in_maps


def _run(x, W, b, trace=False, tmpdir=None):
    from concourse.bass_utils import run_bass_kernel_spmd

    nc = _get_nc()
    res = run_bass_kernel_spmd(
        nc,
        _make_in_maps(x, W, b),
        core_ids=list(range(NCORES)),
        trace=trace,
        tmpdir=tmpdir,
    )
    out = np.concatenate(
        [np.asarray(res.results[i]["out"]) for i in range(NCORES)], axis=0
    )
    return out, res


def kernel(**inputs):
    out, _ = _run(inputs["x"], inputs["W"], inputs["b"])
    return out


def run_profiled(tmpdir=None, **inputs):
    out, res = _run(inputs["x"], inputs["W"], inputs["b"], trace=True, tmpdir=tmpdir)
    return out, res
